# revision 2
# baseline (speedup 1.0000x reference)
"""SAGAN self-attention block on 8 TRN2 NeuronCores (v2: fp8 DoubleRow + wide EXP).

Reference (per batch element b, N = H*W = 4096, C = 512, D = 64):
    f = x @ Wf + bf ; g = x @ Wg + bg ; h = x @ Wh + bh      # [N, D]
    s = f @ g.T                                              # [N, N]
    attn = softmax(s, axis=-1)
    ctx = attn @ h                                           # [N, D]
    o = (gamma * ctx) @ Wv + bv + x                          # [N, C]

Sharding: data-parallel over batch B=8 -> one batch element per core, no
collectives. Weights replicated.

Device algorithm (per core):
  - host passes xT (x transposed, fp8 e4m3) so no on-device transposes;
    x rows (f32) are DMA'd just-in-time for the residual add only.
  - all projection weights are scaled x16 on host and quantized to fp8 e4m3;
    projections run as fp8 DoubleRow matmuls (2 c-tiles of 128 contracted per
    instruction at 0.5 cycles/row).  f' = 16f, g' = 16g land in PSUM;
    DVE adds biases and writes FT8/GT8 (fp8, k-tile 1 zero-padded).
  - QK: s' = g'.T f' = 256 s via fp8 DoubleRow (K=64 real + 64 zero pad),
    one [128,512] PSUM tile per m-tile, grouped 3 m-tiles per 1536-wide
    PSUM tile.
  - unnormalized softmax: ep = exp(s'/256) on ScalarE (scale=1/256 is free),
    one 1536-wide activation per m-group -> SBUF bf16.  No max subtraction:
    |s| < ~80 so exp fits f32/bf16.
  - PV: ctxT[65, n-chunk] += haug[m-tile].T @ ep (bf16, K=128), where
    haug[m,:] = [h_m, 1.0]; row 64 accumulates the softmax denominator.
  - epilogue per n-chunk: ct = bf16(ctxT); denominator row transposed via
    tiny SBUF->SBUF DMA; out[n,:] = (ct[:,n].T @ [gamma*Wv ; bv]) * (1/den_n)
    + x[n,:] with the multiply+residual fused in one DVE scalar_tensor_tensor.
"""

import numpy as np
import ml_dtypes

BF16 = ml_dtypes.bfloat16
E4M3 = ml_dtypes.float8_e4m3

B, HH, WW, C = 8, 64, 64, 512
D = C // 8          # 64
N_FULL = HH * WW    # 4096
P = 128
CC = C // P         # 4  (c-chunks of 128)
WSCALE = 16.0       # host-side weight prescale (undone via exp scale / haug scale)

_CACHE: dict = {}


def _groups():
    """m-tile groups per n-chunk: 10 triples + one pair (32 tiles)."""
    gs = [list(range(3 * k, 3 * k + 3)) for k in range(10)]
    gs.append([30, 31])
    return gs


def _build(n: int, h_bias_zero: bool = False):
    import concourse.mybir as mybir
    from concourse import bacc
    from concourse.tile import TileContext

    f32 = mybir.dt.float32
    bf16 = mybir.dt.bfloat16
    fp8 = mybir.dt.float8e4
    ADD = mybir.AluOpType.add
    MULT = mybir.AluOpType.mult
    EXP = mybir.ActivationFunctionType.Exp
    DR = mybir.MatmulPerfMode.DoubleRow

    n_tiles = n // P        # 32
    nch = n // 512          # 8

    nc = bacc.Bacc("TRN2", target_bir_lowering=False, debug=False)

    x_d = nc.dram_tensor("x", [n, C], f32, kind="ExternalInput")
    xt8_d = nc.dram_tensor("xt8", [P, CC, n], fp8, kind="ExternalInput")
    wfg8_d = nc.dram_tensor("wfg8", [P, 2, 2, 2 * D], fp8, kind="ExternalInput")
    wh8_d = nc.dram_tensor("wh8", [P, 2, 2, D], fp8, kind="ExternalInput")
    bfg_d = nc.dram_tensor("bfg", [P, 1], f32, kind="ExternalInput")   # 16*[bf;bg]
    if not h_bias_zero:
        bh_d = nc.dram_tensor("bhp", [1, D], bf16, kind="ExternalInput")  # 16*bh
        on_d = nc.dram_tensor("onesp", [1, P], bf16, kind="ExternalInput")
    wv_d = nc.dram_tensor("wv", [D + 1, C], bf16, kind="ExternalInput")
    out_d = nc.dram_tensor("out", [n, C], f32, kind="ExternalOutput")

    x_t = x_d.rearrange("(i p) c -> i p c", p=P)
    o_t = out_d.rearrange("(i p) c -> i p c", p=P)

    groups = _groups()

    with TileContext(nc) as tc:
        with (
            tc.tile_pool(name="const", bufs=1) as cpool,
            tc.tile_pool(name="big", bufs=1) as bigpool,
            tc.tile_pool(name="ep", bufs=3) as epool,
            tc.tile_pool(name="ct", bufs=2) as ctpool,
            tc.tile_pool(name="os", bufs=4) as opool,
            tc.tile_pool(name="xr", bufs=8) as xrpool,
            tc.tile_pool(name="sm", bufs=4) as smpool,
            tc.tile_pool(name="psA", bufs=2, space="PSUM") as psA,
            tc.tile_pool(name="psB", bufs=1, space="PSUM") as psB,
            tc.tile_pool(name="psC", bufs=1, space="PSUM") as psC,
        ):
            # ---- replicated constants -> SBUF
            wfg_sb = cpool.tile([P, 2, 2, 2 * D], fp8)
            nc.sync.dma_start(wfg_sb, wfg8_d[:, :, :, :])
            wh_sb = cpool.tile([P, 2, 2, D], fp8)
            nc.sync.dma_start(wh_sb, wh8_d[:, :, :, :])
            bfg_sb = cpool.tile([P, 1], f32)
            nc.sync.dma_start(bfg_sb, bfg_d[:, :])
            if not h_bias_zero:
                bh_sb = cpool.tile([1, D], bf16)
                nc.sync.dma_start(bh_sb, bh_d[:, :])
                ones_sb = cpool.tile([1, P], bf16)
                nc.sync.dma_start(ones_sb, on_d[:, :])
            wv_sb = cpool.tile([D + 1, C], bf16)
            nc.sync.dma_start(wv_sb, wv_d[:, :])

            # ---- persistent SBUF tensors
            xt8 = bigpool.tile([P, CC, n], fp8)          # x.T (c on partitions)
            FT8 = bigpool.tile([D, 2, n], fp8)           # f'.T, k-tile 1 = 0
            GT8 = bigpool.tile([D, 2, n], fp8)           # g'.T, k-tile 1 = 0
            GS = bigpool.tile([P, n], fp8)               # staging for g (rows 64:128)
            haug = bigpool.tile([P, n_tiles, D + 1], bf16)
            nc.gpsimd.memset(FT8[:, 1, :], 0.0)
            nc.gpsimd.memset(GT8[:, 1, :], 0.0)
            nc.gpsimd.memset(haug[:, :, D:D + 1], 1.0)

            # ---- prologue: load xT, project f/g (per 512-chunk) and h (per
            # 128-tile) with fp8 DoubleRow matmuls; bias+cast to fp8 on DVE.
            for jc in range(nch):
                sl = slice(jc * 512, (jc + 1) * 512)
                nc.sync.dma_start(xt8[:, :, sl], xt8_d[:, :, sl])
                fgp = psC.tile([P, 512], f32, tag="oc", name=f"fg{jc}")
                for pr in range(2):
                    nc.tensor.matmul(
                        fgp, lhsT=wfg_sb[:, pr, :, :],
                        rhs=xt8[:, 2 * pr:2 * pr + 2, sl],
                        start=(pr == 0), stop=(pr == 1), perf_mode=DR,
                    )
                nc.vector.tensor_scalar(FT8[:, 0, sl], fgp[0:D, :], bfg_sb[0:D], None, ADD)
                nc.vector.tensor_scalar(GS[D:P, sl], fgp[D:P, :], bfg_sb[D:P], None, ADD)
                nc.sync.dma_start(GT8[:, 0, sl], GS[D:P, sl])
                for i in range(4 * jc, 4 * jc + 4):
                    hp = psC.tile([P, D], f32, tag="oc", name=f"hp{i}")
                    for pr in range(2):
                        nc.tensor.matmul(
                            hp, lhsT=xt8[:, 2 * pr:2 * pr + 2, i * P:(i + 1) * P],
                            rhs=wh_sb[:, pr, :, :],
                            start=(pr == 0),
                            stop=(pr == 1 and h_bias_zero), perf_mode=DR,
                        )
                    if not h_bias_zero:
                        nc.tensor.matmul(
                            hp, lhsT=ones_sb, rhs=bh_sb, start=False, stop=True)
                    nc.vector.tensor_scalar(
                        haug[:, i, 0:D], hp, 1.0 / WSCALE, None, MULT)

            # ---- attention main loop: n-chunks of 512, m-tiles in groups of 3
            for jc in range(nch):
                sl = slice(jc * 512, (jc + 1) * 512)
                for t in range(4):
                    it = jc * 4 + t
                    xr = xrpool.tile([P, C], f32, tag="xr", name=f"xr{it}")
                    nc.sync.dma_start(xr, x_t[it])
                    if t == 0:
                        xrs = {}
                    xrs[t] = xr
                ctx = psB.tile([D + 1, 512], f32, tag="ctx")
                first = True
                for grp in groups:
                    w = 512 * len(grp)
                    sp = psA.tile([P, 1536], f32, tag="sp")
                    for q, i in enumerate(grp):
                        nc.tensor.matmul(
                            sp[:, q * 512:(q + 1) * 512],
                            lhsT=GT8[:, :, i * P:(i + 1) * P],
                            rhs=FT8[:, :, sl],
                            start=True, stop=True, perf_mode=DR,
                        )
                    ep = epool.tile([P, 1536], bf16, tag="ep")
                    nc.scalar.activation(
                        ep[:, 0:w], sp[:, 0:w], EXP, scale=1.0 / (WSCALE * WSCALE))
                    for q, i in enumerate(grp):
                        nc.tensor.matmul(
                            ctx, lhsT=haug[:, i, :],
                            rhs=ep[:, q * 512:(q + 1) * 512],
                            start=first, stop=(i == n_tiles - 1),
                        )
                        first = False

                # ---- epilogue for this n-chunk (4 subtiles of 128 rows)
                ct = ctpool.tile([D + 1, 512], bf16, tag="ct")
                nc.vector.tensor_copy(out=ct, in_=ctx)
                for t in range(4):
                    it = jc * 4 + t
                    tsl = slice(t * P, (t + 1) * P)
                    rcT = smpool.tile([P, 1], bf16, tag="rct")
                    nc.sync.dma_start(rcT, ct[D:D + 1, tsl])
                    rc = smpool.tile([P, 1], f32, tag="rc")
                    nc.vector.reciprocal(rc, rcT)
                    op = psC.tile([P, C], f32, tag="oc", name=f"op{it}")
                    nc.tensor.matmul(op, lhsT=ct[:, tsl], rhs=wv_sb, start=True, stop=True)
                    osb = opool.tile([P, C], f32, tag="os")
                    nc.vector.scalar_tensor_tensor(
                        out=osb, in0=op, scalar=rc, in1=xrs[t], op0=MULT, op1=ADD)
                    if jc == nch - 1:
                        # the final chunk's stores are the kernel tail: split
                        # them across two queues to halve the drain latency
                        nc.sync.dma_start(o_t[it][0:D, :], osb[0:D, :])
                        nc.sync.dma_start(o_t[it][D:P, :], osb[D:P, :])
                    else:
                        nc.sync.dma_start(o_t[it], osb)

    nc.compile()
    return nc


def get_program(n: int = N_FULL, h_bias_zero: bool = False):
    key = (n, h_bias_zero)
    if key not in _CACHE:
        _CACHE[key] = _build(n, h_bias_zero)
    return _CACHE[key]


def make_weight_maps(Wf, bf, Wg, bg, Wh, bh, Wv, bv, gamma, h_bias_zero=False):
    """Host-side layout prep of the tiny replicated weights."""
    wv_aug = np.concatenate(
        [np.float32(gamma) * np.asarray(Wv, np.float32),
         np.asarray(bv, np.float32)[None, :]], axis=0)
    bfg = WSCALE * np.concatenate(
        [np.asarray(bf, np.float32), np.asarray(bg, np.float32)]).reshape(P, 1)
    wfg = WSCALE * np.concatenate(
        [np.asarray(Wf, np.float32), np.asarray(Wg, np.float32)], axis=1)
    # c index decomposition: c = (2*pr + tile)*128 + p  ->  [p, pr, tile, d]
    wfg8 = wfg.astype(E4M3).reshape(2, 2, P, 2 * D).transpose(2, 0, 1, 3)
    wh8 = (WSCALE * np.asarray(Wh, np.float32)).astype(E4M3) \
        .reshape(2, 2, P, D).transpose(2, 0, 1, 3)
    maps = {
        "wfg8": np.ascontiguousarray(wfg8),
        "wh8": np.ascontiguousarray(wh8),
        "bfg": np.ascontiguousarray(bfg),
        "bhp": np.ascontiguousarray(
            (WSCALE * np.asarray(bh, np.float32)).astype(BF16).reshape(1, D)),
        "onesp": np.ones((1, P), dtype=BF16),
        "wv": np.ascontiguousarray(wv_aug.astype(BF16)),
    }
    if h_bias_zero:
        del maps["bhp"], maps["onesp"]
    return maps


def make_x_maps(xf_b):
    """Per-core x layouts: residual rows (f32) + transposed fp8 [p, cc, n]."""
    x = np.ascontiguousarray(xf_b, dtype=np.float32)
    xt8 = x.T.astype(E4M3).reshape(CC, P, x.shape[0]).transpose(1, 0, 2)
    return {"x": x, "xt8": np.ascontiguousarray(xt8)}


def kernel(x, Wf, bf, Wg, bg, Wh, bh, Wv, bv, gamma):
    from concourse.bass_utils import run_bass_kernel_spmd

    x = np.asarray(x, np.float32)
    b, hh, ww, c = x.shape
    n = hh * ww
    assert (b, c) == (B, C)

    hbz = bool(np.all(np.asarray(bh) == 0))
    nc = get_program(n, hbz)
    base = make_weight_maps(Wf, bf, Wg, bg, Wh, bh, Wv, bv, gamma, hbz)
    xf = x.reshape(b, n, c)
    in_maps = [dict(base, **make_x_maps(xf[i])) for i in range(b)]

    res = run_bass_kernel_spmd(nc, in_maps, core_ids=list(range(b)))
    out = np.stack([res.results[i]["out"] for i in range(b)], axis=0)
    return np.ascontiguousarray(out.reshape(b, hh, ww, c).astype(np.float32))


# revision 3
# speedup vs baseline: 1.4920x; 1.4920x over previous
"""SAGAN self-attention block on 8 TRN2 NeuronCores (v3).

Reference (per batch element b, N = H*W = 4096, C = 512, D = 64):
    f = x @ Wf + bf ; g = x @ Wg + bg ; h = x @ Wh + bh      # [N, D]
    s = f @ g.T                                              # [N, N]
    attn = softmax(s, axis=-1)
    ctx = attn @ h                                           # [N, D]
    o = (gamma * ctx) @ Wv + bv + x                          # [N, C]

Sharding: data-parallel over batch B=8 -> one batch element per core, no
collectives. Weights replicated.

Device algorithm (per core), matmuls in bf16 with f32 PSUM accumulation:
  - host passes xT (x transposed, bf16) so no on-device PE transposes;
    x rows (f32) are DMA'd just-in-time for the residual add only.
  - f and g projected in ONE matmul chain per 512-chunk using stacked
    [Wf|Wg] weights (M=128): fT lands on PSUM partitions 0:64, gT on
    64:128; DVE adds biases writing FT2/GT2 halves, which are then
    mirrored into the other partition half via SBUF->SBUF DMA so QK
    pairs can row-pack (K=64 streams 2 cols/cycle).
  - h_aug[m, :] = [x@Wh + bh, 1.0]  -> [4096, 65] bf16 (m on partitions).
  - unnormalized softmax (no max subtraction: |s| <~ 80 fits f32/bf16):
    m-tiles in groups of 3 -> one [128, 1536] PSUM tile, ONE wide EXP
    (ScalarE) per group -> SBUF bf16; PV accumulates ctxT[65, n-chunk]
    += haug[i].T @ ep (K=128); row 64 = softmax denominator (ones col).
  - epilogue per n-chunk: ct = bf16(ctxT); denominator row transposed to
    [128,1] via tiny SBUF->SBUF DMA; out = (ct.T @ [gamma*Wv ; bv]) *
    (1/den) + x fused in one DVE scalar_tensor_tensor.
"""

import numpy as np
import ml_dtypes

BF16 = ml_dtypes.bfloat16

B, HH, WW, C = 8, 64, 64, 512
D = C // 8          # 64
N_FULL = HH * WW    # 4096
P = 128
CC = C // P         # 4  (c-chunks of 128)

_CACHE: dict = {}


def _groups(n_tiles):
    """m-tile groups per n-chunk: triples + remainder (e.g. 10x3 + 1x2)."""
    gs = []
    i = 0
    while n_tiles - i >= 3:
        if n_tiles - i == 4:
            break
        gs.append([i, i + 1, i + 2])
        i += 3
    while i < n_tiles:
        gs.append(list(range(i, min(i + 2, n_tiles))))
        i += 2
    return gs


def _build(n: int, h_bias_zero: bool = False):
    import concourse.mybir as mybir
    from concourse import bacc
    from concourse.tile import TileContext

    f32 = mybir.dt.float32
    bf16 = mybir.dt.bfloat16
    ADD = mybir.AluOpType.add
    MULT = mybir.AluOpType.mult
    EXP = mybir.ActivationFunctionType.Exp

    n_tiles = n // P        # 32
    nch = n // 512          # 8

    nc = bacc.Bacc("TRN2", target_bir_lowering=False, debug=False)

    x_d = nc.dram_tensor("x", [n, C], f32, kind="ExternalInput")
    xt_d = nc.dram_tensor("xt", [P, CC, n], bf16, kind="ExternalInput")
    wfg_d = nc.dram_tensor("wfg", [P, CC, 2 * D], bf16, kind="ExternalInput")
    wh_d = nc.dram_tensor("wh", [P, CC, D], bf16, kind="ExternalInput")
    bfg_d = nc.dram_tensor("bfg", [P, 1], f32, kind="ExternalInput")   # [bf;bg]
    if not h_bias_zero:
        bh_d = nc.dram_tensor("bhp", [1, D], bf16, kind="ExternalInput")
        on_d = nc.dram_tensor("onesp", [1, P], bf16, kind="ExternalInput")
    wv_d = nc.dram_tensor("wv", [D + 1, C], bf16, kind="ExternalInput")
    out_d = nc.dram_tensor("out", [n, C], f32, kind="ExternalOutput")

    x_t = x_d.rearrange("(i p) c -> i p c", p=P)
    o_t = out_d.rearrange("(i p) c -> i p c", p=P)

    groups = _groups(n_tiles)

    with TileContext(nc) as tc:
        with (
            tc.tile_pool(name="const", bufs=1) as cpool,
            tc.tile_pool(name="big", bufs=1) as bigpool,
            tc.tile_pool(name="ep", bufs=3) as epool,
            tc.tile_pool(name="ct", bufs=2) as ctpool,
            tc.tile_pool(name="os", bufs=4) as opool,
            tc.tile_pool(name="xr", bufs=8) as xrpool,
            tc.tile_pool(name="sm", bufs=4) as smpool,
            tc.tile_pool(name="psA", bufs=2, space="PSUM") as psA,
            tc.tile_pool(name="psB", bufs=1, space="PSUM") as psB,
            tc.tile_pool(name="psC", bufs=1, space="PSUM") as psC,
        ):
            # ---- replicated constants -> SBUF
            wfg_sb = cpool.tile([P, CC, 2 * D], bf16)
            nc.sync.dma_start(wfg_sb, wfg_d[:, :, :])
            wh_sb = cpool.tile([P, CC, D], bf16)
            nc.sync.dma_start(wh_sb, wh_d[:, :, :])
            bfg_sb = cpool.tile([P, 1], f32)
            nc.sync.dma_start(bfg_sb, bfg_d[:, :])
            if not h_bias_zero:
                bh_sb = cpool.tile([1, D], bf16)
                nc.sync.dma_start(bh_sb, bh_d[:, :])
                ones_sb = cpool.tile([1, P], bf16)
                nc.sync.dma_start(ones_sb, on_d[:, :])
            wv_sb = cpool.tile([D + 1, C], bf16)
            nc.sync.dma_start(wv_sb, wv_d[:, :])

            # ---- persistent SBUF tensors
            xt = bigpool.tile([P, CC, n], bf16)          # x.T (c on partitions)
            FT2 = bigpool.tile([P, n], bf16)             # f.T duplicated in both halves
            GT2 = bigpool.tile([P, n], bf16)             # g.T duplicated in both halves
            haug = bigpool.tile([P, n_tiles, D + 1], bf16)
            nc.gpsimd.memset(haug[:, :, D:D + 1], 1.0)

            # ---- prologue: load xT, project f/g per 512-chunk and h per
            # 128-tile; bias add + bf16 cast on DVE; mirror halves via DMA.
            for jc in range(nch):
                sl = slice(jc * 512, (jc + 1) * 512)
                nc.sync.dma_start(xt[:, :, sl], xt_d[:, :, sl])
                fgp = psC.tile([P, 512], f32, tag="oc", name=f"fg{jc}")
                for cc in range(CC):
                    nc.tensor.matmul(
                        fgp, lhsT=wfg_sb[:, cc, :], rhs=xt[:, cc, sl],
                        start=(cc == 0), stop=(cc == CC - 1),
                    )
                nc.vector.tensor_scalar(FT2[0:D, sl], fgp[0:D, :], bfg_sb[0:D], None, ADD)
                nc.vector.tensor_scalar(GT2[D:P, sl], fgp[D:P, :], bfg_sb[D:P], None, ADD)
                nc.sync.dma_start(FT2[D:P, sl], FT2[0:D, sl])
                nc.sync.dma_start(GT2[0:D, sl], GT2[D:P, sl])
                for i in range(4 * jc, 4 * jc + 4):
                    hp = psC.tile([P, D], f32, tag="oc", name=f"hp{i}")
                    for cc in range(CC):
                        nc.tensor.matmul(
                            hp, lhsT=xt[:, cc, i * P:(i + 1) * P], rhs=wh_sb[:, cc, :],
                            start=(cc == 0), stop=(h_bias_zero and cc == CC - 1),
                        )
                    if not h_bias_zero:
                        nc.tensor.matmul(
                            hp, lhsT=ones_sb, rhs=bh_sb, start=False, stop=True)
                    nc.vector.tensor_copy(out=haug[:, i, 0:D], in_=hp)

            # ---- attention main loop: n-chunks of 512, m-tiles in groups of 3
            for jc in range(nch):
                sl = slice(jc * 512, (jc + 1) * 512)
                xrs = {}
                for t in range(4):
                    it = jc * 4 + t
                    xr = xrpool.tile([P, C], f32, tag="xr", name=f"xr{it}")
                    nc.sync.dma_start(xr, x_t[it])
                    xrs[t] = xr
                ctx = psB.tile([D + 1, 512], f32, tag="ctx")
                first = True
                for grp in groups:
                    w = 512 * len(grp)
                    sp = psA.tile([P, 1536], f32, tag="sp")
                    for q, i in enumerate(grp):
                        # row-pack QK by m-tile parity: even tiles read the
                        # lower halves of GT2/FT2, odd tiles the upper mirrors
                        hb = (i % 2) * D
                        nc.tensor.matmul(
                            sp[:, q * 512:(q + 1) * 512],
                            lhsT=GT2[hb:hb + D, i * P:(i + 1) * P],
                            rhs=FT2[hb:hb + D, sl],
                            start=True, stop=True, tile_position=(hb, 0),
                        )
                    ep = epool.tile([P, 1536], bf16, tag="ep")
                    nc.scalar.activation(ep[:, 0:w], sp[:, 0:w], EXP)
                    for q, i in enumerate(grp):
                        nc.tensor.matmul(
                            ctx, lhsT=haug[:, i, :],
                            rhs=ep[:, q * 512:(q + 1) * 512],
                            start=first, stop=(i == n_tiles - 1),
                        )
                        first = False

                # ---- epilogue for this n-chunk (4 subtiles of 128 rows)
                ct = ctpool.tile([D + 1, 512], bf16, tag="ct")
                nc.vector.tensor_copy(out=ct, in_=ctx)
                for t in range(4):
                    it = jc * 4 + t
                    tsl = slice(t * P, (t + 1) * P)
                    rcT = smpool.tile([P, 1], bf16, tag="rct")
                    nc.sync.dma_start(rcT, ct[D:D + 1, tsl])
                    rc = smpool.tile([P, 1], f32, tag="rc")
                    nc.vector.reciprocal(rc, rcT)
                    op = psC.tile([P, C], f32, tag="oc", name=f"op{it}")
                    nc.tensor.matmul(op, lhsT=ct[:, tsl], rhs=wv_sb, start=True, stop=True)
                    osb = opool.tile([P, C], f32, tag="os")
                    nc.vector.scalar_tensor_tensor(
                        out=osb, in0=op, scalar=rc, in1=xrs[t], op0=MULT, op1=ADD)
                    if jc == nch - 1:
                        # the final chunk's stores are the kernel tail: split
                        # them across two queues to halve the drain latency
                        nc.sync.dma_start(o_t[it][0:D, :], osb[0:D, :])
                        nc.sync.dma_start(o_t[it][D:P, :], osb[D:P, :])
                    else:
                        nc.sync.dma_start(o_t[it], osb)

    nc.compile()
    return nc


def get_program(n: int = N_FULL, h_bias_zero: bool = False):
    key = (n, h_bias_zero)
    if key not in _CACHE:
        _CACHE[key] = _build(n, h_bias_zero)
    return _CACHE[key]


def make_weight_maps(Wf, bf, Wg, bg, Wh, bh, Wv, bv, gamma, h_bias_zero=False):
    """Host-side layout prep of the tiny replicated weights."""
    wv_aug = np.concatenate(
        [np.float32(gamma) * np.asarray(Wv, np.float32),
         np.asarray(bv, np.float32)[None, :]], axis=0)
    bfg = np.concatenate(
        [np.asarray(bf, np.float32), np.asarray(bg, np.float32)]).reshape(P, 1)
    wfg = np.concatenate(
        [np.asarray(Wf, np.float32), np.asarray(Wg, np.float32)], axis=1)
    # c index decomposition: c = cc*128 + p  ->  [p, cc, d]
    maps = {
        "wfg": np.ascontiguousarray(
            wfg.astype(BF16).reshape(CC, P, 2 * D).transpose(1, 0, 2)),
        "wh": np.ascontiguousarray(
            np.asarray(Wh, np.float32).astype(BF16).reshape(CC, P, D).transpose(1, 0, 2)),
        "bfg": np.ascontiguousarray(bfg),
        "bhp": np.ascontiguousarray(
            np.asarray(bh, np.float32).astype(BF16).reshape(1, D)),
        "onesp": np.ones((1, P), dtype=BF16),
        "wv": np.ascontiguousarray(wv_aug.astype(BF16)),
    }
    if h_bias_zero:
        del maps["bhp"], maps["onesp"]
    return maps


def make_x_maps(xf_b):
    """Per-core x layouts: residual rows (f32) + transposed bf16 [p, cc, n]."""
    x = np.ascontiguousarray(xf_b, dtype=np.float32)
    xt = x.T.astype(BF16).reshape(CC, P, x.shape[0]).transpose(1, 0, 2)
    return {"x": x, "xt": np.ascontiguousarray(xt)}


def kernel(x, Wf, bf, Wg, bg, Wh, bh, Wv, bv, gamma):
    from concourse.bass_utils import run_bass_kernel_spmd

    x = np.asarray(x, np.float32)
    b, hh, ww, c = x.shape
    n = hh * ww
    assert (b, c) == (B, C)

    hbz = bool(np.all(np.asarray(bh) == 0))
    nc = get_program(n, hbz)
    base = make_weight_maps(Wf, bf, Wg, bg, Wh, bh, Wv, bv, gamma, hbz)
    xf = x.reshape(b, n, c)
    in_maps = [dict(base, **make_x_maps(xf[i])) for i in range(b)]

    res = run_bass_kernel_spmd(nc, in_maps, core_ids=list(range(b)))
    out = np.stack([res.results[i]["out"] for i in range(b)], axis=0)
    return np.ascontiguousarray(out.reshape(b, hh, ww, c).astype(np.float32))
